# revision 25
# baseline (speedup 1.0000x reference)
import sys

sys.path.insert(0, "/opt/trn_rl_repo")

import numpy as np
import ml_dtypes

import concourse.bass as bass
import concourse.bacc as bacc
import concourse.mybir as mybir
from concourse.tile import TileContext
from concourse.bass_utils import run_bass_kernel_spmd

P = 128          # partitions
BT = 512         # batch-tile (free dim)
G = 4            # batch groups packed into 128 partitions for the GRU
NCORES = 8
B, S, H, A = 131072, 256, 512, 32
BC = B // NCORES           # 16384 rows per core
MACRO = G * BT             # 2048 rows per GRU macro-tile
NM = BC // MACRO           # 8 macro-tiles per core

FP32 = mybir.dt.float32
BF16 = mybir.dt.bfloat16
AF = mybir.ActivationFunctionType
OP = mybir.AluOpType
BF = ml_dtypes.bfloat16

_CACHE = {}


def _build(nsteps: int) -> bass.Bass:
    nc = bacc.Bacc("TRN2", target_bir_lowering=False, debug=False,
                   num_devices=NCORES)

    xT = nc.dram_tensor("xT", [S, BC], BF16, kind="ExternalInput")
    w1t = nc.dram_tensor("w1t", [S, H], BF16, kind="ExternalInput")
    w2t = nc.dram_tensor("w2t", [H, H], BF16, kind="ExternalInput")
    wmt = nc.dram_tensor("wmt", [H, A], BF16, kind="ExternalInput")
    b1d = nc.dram_tensor("b1d", [P, 4], FP32, kind="ExternalInput")
    b2d = nc.dram_tensor("b2d", [P, 4], FP32, kind="ExternalInput")
    bmd = nc.dram_tensor("bmd", [P, 1], FP32, kind="ExternalInput")
    # i-side gate weights with bias folded in via a constant-1 ninth row
    lrid = nc.dram_tensor("lrid", [9, P], BF16, kind="ExternalInput")
    luid = nc.dram_tensor("luid", [9, P], BF16, kind="ExternalInput")
    lnid = nc.dram_tensor("lnid", [9, P], BF16, kind="ExternalInput")
    lrhd = nc.dram_tensor("lrhd", [P, P], BF16, kind="ExternalInput")
    luhd = nc.dram_tensor("luhd", [P, P], BF16, kind="ExternalInput")
    lnhd = nc.dram_tensor("lnhd", [P, P], BF16, kind="ExternalInput")
    lwd = nc.dram_tensor("lwd", [P, 2 * G], BF16, kind="ExternalInput")
    bnhd = nc.dram_tensor("bnhd", [P, 1], FP32, kind="ExternalInput")
    bwd = nc.dram_tensor("bwd", [2 * G, 1], FP32, kind="ExternalInput")
    wpinit = nc.dram_tensor("wpinit", [9, BT], BF16, kind="ExternalInput")
    outd = nc.dram_tensor("outd", [nsteps, NM, 2 * G, BT], BF16,
                          kind="ExternalOutput")

    xv = xT.rearrange("(kb p) b -> p kb b", p=P)              # [128, 2, BC]

    with TileContext(nc) as tc:
        with (
            tc.tile_pool(name="const", bufs=1) as const,
            tc.tile_pool(name="state", bufs=1) as state,
            tc.tile_pool(name="xp", bufs=3) as xp,
            tc.tile_pool(name="hp", bufs=2) as hp,
            tc.tile_pool(name="rup", bufs=3) as rup,
            tc.tile_pool(name="ntp", bufs=2) as ntp,
            tc.tile_pool(name="p1p", bufs=2) as p1p,
            tc.tile_pool(name="p2p", bufs=3) as p2p,
            tc.tile_pool(name="dp", bufs=2) as dp,
            tc.tile_pool(name="mps", bufs=2, space="PSUM") as mps,
            tc.tile_pool(name="zps", bufs=1, space="PSUM") as zps,
            tc.tile_pool(name="gps", bufs=1, space="PSUM") as gps,
            tc.tile_pool(name="nhp", bufs=1, space="PSUM") as nhp,
            tc.tile_pool(name="nip", bufs=1, space="PSUM") as nip,
            tc.tile_pool(name="wpp", bufs=1, space="PSUM") as wpp,
        ):
            w1s = const.tile([P, 2, H], BF16)
            nc.sync.dma_start(w1s[:], w1t.rearrange("(kb p) f -> p kb f", p=P))
            b1s = const.tile([P, 4], FP32)
            nc.sync.dma_start(b1s[:], b1d[:])
            w2s = const.tile([P, 4, H], BF16)
            nc.sync.dma_start(w2s[:], w2t.rearrange("(kb p) f -> p kb f", p=P))
            b2s = const.tile([P, 4], FP32)
            nc.sync.dma_start(b2s[:], b2d[:])
            wms = const.tile([P, 4, A], BF16)
            nc.sync.dma_start(wms[:], wmt.rearrange("(kb p) f -> p kb f", p=P))
            bms = const.tile([P, 1], FP32)
            nc.sync.dma_start(bms[:], bmd[:])

            def load_gru_consts():
                lris = const.tile([9, P], BF16, name="lris")
                nc.sync.dma_start(lris[:], lrid[:])
                luis = const.tile([9, P], BF16, name="luis")
                nc.sync.dma_start(luis[:], luid[:])
                lnis = const.tile([9, P], BF16, name="lnis")
                nc.sync.dma_start(lnis[:], lnid[:])
                lrhs = const.tile([P, P], BF16, name="lrhs")
                nc.sync.dma_start(lrhs[:], lrhd[:])
                luhs = const.tile([P, P], BF16, name="luhs")
                nc.sync.dma_start(luhs[:], luhd[:])
                lnhs = const.tile([P, P], BF16, name="lnhs")
                nc.sync.dma_start(lnhs[:], lnhd[:])
                lws = const.tile([P, 2 * G], BF16, name="lws")
                nc.sync.dma_start(lws[:], lwd[:])
                bnhs = const.tile([P, 1], FP32, name="bnhs")
                nc.sync.dma_start(bnhs[:], bnhd[:])
                bws = const.tile([2 * G, 1], FP32, name="bws")
                nc.sync.dma_start(bws[:], bwd[:])
                wps = []
                for m in range(NM):
                    wt = state.tile([9, BT], BF16, tag=f"WP{m}", name=f"WP{m}")
                    wps.append(wt)
                    nc.sync.dma_start(wt[:], wpinit[:])
                return lris, luis, lnis, lrhs, luhs, lnhs, lws, bnhs, bws, wps

            Z = []
            for m in range(NM):
                zt = state.tile([P, BT], BF16, tag=f"Z{m}", name=f"Z{m}")
                Z.append(zt)

            # ------------- emission helpers -------------
            psZs = [None] * NM

            def emit_mlp_group(m, g):
                c0 = m * MACRO + g * BT
                X = xp.tile([P, 2, BT], BF16, tag="X", name="X")
                nc.sync.dma_start(X[:], xv[:, :, c0:c0 + BT])
                H1 = hp.tile([P, 4, BT], BF16, tag="H1", name="H1")
                for f in range(4):
                    ps = mps.tile([P, BT], FP32, tag="mm", name="ps")
                    nc.tensor.matmul(ps[:], w1s[:, 0, f * P:(f + 1) * P],
                                     X[:, 0, :], start=True, stop=False)
                    nc.tensor.matmul(ps[:], w1s[:, 1, f * P:(f + 1) * P],
                                     X[:, 1, :], start=False, stop=True)
                    if f % 2 == 0:
                        nc.scalar.activation(H1[:, f, :], ps[:], AF.Relu,
                                             bias=b1s[:, f:f + 1])
                    else:
                        nc.vector.tensor_scalar(H1[:, f, :], ps[:],
                                                b1s[:, f:f + 1], 0.0,
                                                OP.add, OP.max)
                H2 = hp.tile([P, 4, BT], BF16, tag="H2", name="H2")
                for f in range(4):
                    ps = mps.tile([P, BT], FP32, tag="mm", name="ps")
                    for k in range(4):
                        nc.tensor.matmul(ps[:], w2s[:, k, f * P:(f + 1) * P],
                                         H1[:, k, :], start=(k == 0),
                                         stop=(k == 3))
                    if f % 2 == 0:
                        nc.scalar.activation(H2[:, f, :], ps[:], AF.Relu,
                                             bias=b2s[:, f:f + 1])
                    else:
                        nc.vector.tensor_scalar(H2[:, f, :], ps[:],
                                                b2s[:, f:f + 1], 0.0,
                                                OP.add, OP.max)
                if g == 0:
                    psZs[m] = zps.tile([P, BT], FP32, tag="psZ", name="psZ")
                for k in range(4):
                    nc.tensor.matmul(psZs[m][g * A:(g + 1) * A, :],
                                     wms[:, k, :], H2[:, k, :],
                                     start=(k == 0), stop=(k == 3),
                                     tile_position=(0, g * A))
                if g == G - 1:
                    nc.scalar.activation(Z[m][:], psZs[m][:], AF.Identity,
                                         bias=bms[:, 0:1])

            # GRU iteration state, staged across issue-slots:
            #  A(q): gate matmuls, r/u/v sigmoids, P1/P2, M2 = u*z_old
            #  B(q): NT tanh, M1 = v*nt, z = M1 + M2, psW   (1 slot later)
            #  C(q): wp accumulate + output DMA             (2 slots later)
            class It:
                __slots__ = ("t", "m", "psRU", "psNH", "psNI", "R", "U", "V",
                             "P1", "P2", "M2", "NT", "psW", "pool_m1")

            def emit_A_gates(q):
                m = q.m
                q.psRU = gps.tile([P, 2 * BT], FP32, tag="psRU", name="psRU")
                nc.tensor.matmul(q.psRU[:, 0:BT], lris[:], WPS[m][:],
                                 start=True, stop=False)
                nc.tensor.matmul(q.psRU[:, 0:BT], lrhs[:], Z[m][:],
                                 start=False, stop=True)
                nc.tensor.matmul(q.psRU[:, BT:2 * BT], luis[:], WPS[m][:],
                                 start=True, stop=False)
                nc.tensor.matmul(q.psRU[:, BT:2 * BT], luhs[:], Z[m][:],
                                 start=False, stop=True)
                q.psNH = nhp.tile([P, BT], FP32, tag="psNH", name="psNH")
                nc.tensor.matmul(q.psNH[:], lnhs[:], Z[m][:],
                                 start=True, stop=True)
                q.psNI = nip.tile([P, BT], FP32, tag="psNI", name="psNI")
                nc.tensor.matmul(q.psNI[:], lnis[:], WPS[m][:],
                                 start=True, stop=True)

            def emit_A_rest(q):
                # Act: r and u sigmoids, v = 1-u = sigmoid(-pre_u)
                q.R = rup.tile([P, BT], BF16, tag="R", name="R")
                nc.scalar.activation(q.R[:], q.psRU[:, 0:BT], AF.Sigmoid)
                q.U = rup.tile([P, BT], BF16, tag="U", name="U")
                nc.scalar.activation(q.U[:], q.psRU[:, BT:2 * BT], AF.Sigmoid)
                q.V = rup.tile([P, BT], BF16, tag="V", name="V")
                nc.scalar.activation(q.V[:], q.psRU[:, BT:2 * BT], AF.Sigmoid,
                                     scale=-1.0)
                # DVE: n-gate pre-activation from PSUM
                q.P1 = p1p.tile([P, BT], FP32, tag="P1", name="P1")
                nc.vector.scalar_tensor_tensor(q.P1[:], q.psNH[:],
                                               bnhs[:, 0:1], q.R[:],
                                               OP.add, OP.mult)
                q.P2 = p2p.tile([P, BT], FP32, tag="P2", name="P2")
                nc.vector.scalar_tensor_tensor(q.P2[:], q.psNI[:], 0.0,
                                               q.P1[:], OP.add, OP.add)
                # Pool (off critical path): m2 = u * z_old
                q.M2 = dp.tile([P, BT], BF16, tag="M2", name="M2")
                nc.gpsimd.tensor_tensor(q.M2[:], q.U[:], Z[q.m][:], OP.mult)

            def emit_B(q):
                m = q.m
                q.NT = ntp.tile([P, BT], BF16, tag="NT", name="NT")
                nc.scalar.activation(q.NT[:], q.P2[:], AF.Tanh)
                M1 = dp.tile([P, BT], BF16, tag="M1", name="M1")
                if q.pool_m1:
                    nc.gpsimd.tensor_tensor(M1[:], q.V[:], q.NT[:], OP.mult)
                else:
                    nc.vector.scalar_tensor_tensor(M1[:], q.V[:], 0.0,
                                                   q.NT[:], OP.add, OP.mult)
                nc.vector.scalar_tensor_tensor(Z[m][:], M1[:], 0.0, q.M2[:],
                                               OP.add, OP.add)

            def emit_C(q):
                m = q.m
                q.psW = wpp.tile([2 * G, BT], FP32, tag="psW", name="psW")
                nc.tensor.matmul(q.psW[:], lws[:], Z[m][:],
                                 start=True, stop=True)
                nc.vector.scalar_tensor_tensor(WPS[m][0:8, :], q.psW[:],
                                               bws[:, 0:1], WPS[m][0:8, :],
                                               OP.add, OP.add)
                nc.sync.dma_start(outd[q.t, m, :, :], WPS[m][0:8, :])

            # ------------- merged list schedule -------------
            next_t = [0] * NM
            last_slot = [-10] * NM
            mlp_done = [False] * NM
            # hold back late macros during the merged phase so the drain tail
            # keeps several macros in rotation (hides recurrence latency)
            caps = [nsteps, nsteps, nsteps, nsteps, nsteps - 2,
                    nsteps - 4, nsteps - 7, nsteps]
            merged_phase = [True]
            b_queue = []        # (slot_of_A, It)
            c_queue = []        # (slot_of_B, It)
            slot = 0

            def gru_slot():
                nonlocal slot
                # pop due C first (ready PE/DVE work heads the queues)
                if c_queue and c_queue[0][0] <= slot - 1:
                    emit_C(c_queue.pop(0)[1])
                # new A: greedy pick ready macro with most remaining steps
                pick = -1
                best = 0
                for m in range(NM):
                    if (mlp_done[m] and next_t[m] < caps[m]
                            and last_slot[m] <= slot - 2):
                        rem = nsteps - next_t[m]
                        if rem > best:
                            best = rem
                            pick = m
                qa = None
                if pick >= 0:
                    qa = It()
                    qa.t = next_t[pick]
                    qa.m = pick
                    qa.pool_m1 = merged_phase[0]
                    next_t[pick] += 1
                    last_slot[pick] = slot
                    emit_A_gates(qa)
                # due B (deps all from earlier slots -> engines see ready work)
                if b_queue and b_queue[0][0] <= slot - 1:
                    qb = b_queue.pop(0)[1]
                    emit_B(qb)
                    c_queue.append((slot, qb))
                if qa is not None:
                    emit_A_rest(qa)
                    b_queue.append((slot, qa))
                slot += 1

            for s in range(NM * G):
                emit_mlp_group(s // G, s % G)
                if s == 0:
                    (lris, luis, lnis, lrhs, luhs, lnhs, lws, bnhs, bws,
                     WPS) = load_gru_consts()
                if s % G == G - 1:
                    mlp_done[s // G] = True
                for _ in range(3):
                    gru_slot()
            caps = [nsteps] * NM
            merged_phase[0] = False
            while (any(next_t[m] < nsteps for m in range(NM))
                   or b_queue or c_queue):
                gru_slot()
    nc.compile()
    return nc


LAST_RESULT = None


def kernel(**inputs) -> np.ndarray:
    global LAST_RESULT
    x = np.asarray(inputs["x"], np.float32)
    W1 = np.asarray(inputs["W1"], np.float32)
    b1 = np.asarray(inputs["b1"], np.float32)
    W2 = np.asarray(inputs["W2"], np.float32)
    b2 = np.asarray(inputs["b2"], np.float32)
    Wm = np.asarray(inputs["Wm"], np.float32)
    bm = np.asarray(inputs["bm"], np.float32)
    w_ih = np.asarray(inputs["w_ih"], np.float32)
    w_hh = np.asarray(inputs["w_hh"], np.float32)
    b_ih = np.asarray(inputs["b_ih"], np.float32)
    b_hh = np.asarray(inputs["b_hh"], np.float32)
    Ww = np.asarray(inputs["Ww"], np.float32)
    bw = np.asarray(inputs["bw"], np.float32)
    T = int(inputs["pred_length"])

    I4 = np.eye(G, dtype=np.float32)

    def pack9(wg, bias):
        # [9, 128]: rows 0-7 block-diag i-weights, row 8 the folded bias
        mret = np.zeros((9, P), np.float32)
        mret[0:8, :] = np.kron(I4, wg.T)
        mret[8, :] = np.tile(bias, G)
        return mret.astype(BF)

    common = {
        "w1t": W1.T.astype(BF),
        "w2t": W2.T.astype(BF),
        "wmt": Wm.T.astype(BF),
        "b1d": np.ascontiguousarray(b1.reshape(4, P).T),
        "b2d": np.ascontiguousarray(b2.reshape(4, P).T),
        "bmd": np.tile(bm, G).reshape(P, 1).copy(),
        "lrid": pack9(w_ih[0:A], b_ih[0:A] + b_hh[0:A]),
        "luid": pack9(w_ih[A:2 * A], b_ih[A:2 * A] + b_hh[A:2 * A]),
        "lnid": pack9(w_ih[2 * A:3 * A], b_ih[2 * A:3 * A]),
        "lrhd": np.kron(I4, w_hh[0:A].T).astype(BF),
        "luhd": np.kron(I4, w_hh[A:2 * A].T).astype(BF),
        "lnhd": np.kron(I4, w_hh[2 * A:3 * A].T).astype(BF),
        "lwd": np.kron(I4, Ww.T).astype(BF),
        "bnhd": np.tile(b_hh[2 * A:3 * A], G).reshape(P, 1).copy(),
        "bwd": np.tile(bw, G).reshape(2 * G, 1).copy(),
        "wpinit": np.concatenate(
            [np.zeros((8, BT), np.float32), np.ones((1, BT), np.float32)]
        ).astype(BF),
    }
    xTb = x.T.astype(BF)                     # [S, B]
    in_maps = []
    for i in range(NCORES):
        m = dict(common)
        m["xT"] = np.ascontiguousarray(xTb[:, i * BC:(i + 1) * BC])
        in_maps.append(m)

    if T not in _CACHE:
        _CACHE[T] = _build(T)
    nc = _CACHE[T]
    res = run_bass_kernel_spmd(nc, in_maps, core_ids=list(range(NCORES)))
    LAST_RESULT = res
    parts = []
    for i in range(NCORES):
        o = np.asarray(res.results[i]["outd"]).astype(np.float32)
        # [T, NM, 2G, BT] -> rows m*2048 + g*512 + c, cols 2t+j
        o = o.reshape(T, NM, G, 2, BT).transpose(1, 2, 4, 0, 3)
        parts.append(o.reshape(BC, 2 * T))
    return np.ascontiguousarray(np.concatenate(parts, axis=0))


# revision 26
# speedup vs baseline: 1.2594x; 1.2594x over previous
import sys

sys.path.insert(0, "/opt/trn_rl_repo")

import numpy as np
import ml_dtypes

import concourse.bass as bass
import concourse.bacc as bacc
import concourse.mybir as mybir
from concourse.tile import TileContext
from concourse.bass_utils import run_bass_kernel_spmd

P = 128          # partitions
BT = 512         # batch-tile (free dim)
G = 4            # batch groups packed into 128 partitions for the GRU
NCORES = 8
B, S, H, A = 131072, 256, 512, 32
BC = B // NCORES           # 16384 rows per core
MACRO = G * BT             # 2048 rows per GRU macro-tile
NM = BC // MACRO           # 8 macro-tiles per core

FP32 = mybir.dt.float32
BF16 = mybir.dt.bfloat16
AF = mybir.ActivationFunctionType
OP = mybir.AluOpType
BF = ml_dtypes.bfloat16

_CACHE = {}


def _build(nsteps: int) -> bass.Bass:
    nc = bacc.Bacc("TRN2", target_bir_lowering=False, debug=False,
                   num_devices=NCORES)

    xT = nc.dram_tensor("xT", [S, BC], BF16, kind="ExternalInput")
    w1t = nc.dram_tensor("w1t", [S, H], BF16, kind="ExternalInput")
    w2t = nc.dram_tensor("w2t", [H, H], BF16, kind="ExternalInput")
    wmt = nc.dram_tensor("wmt", [H, A], BF16, kind="ExternalInput")
    b1d = nc.dram_tensor("b1d", [P, 4], FP32, kind="ExternalInput")
    b2d = nc.dram_tensor("b2d", [P, 4], FP32, kind="ExternalInput")
    bmd = nc.dram_tensor("bmd", [P, 1], FP32, kind="ExternalInput")
    # i-side gate weights with bias folded in via a constant-1 ninth row
    lrid = nc.dram_tensor("lrid", [9, P], BF16, kind="ExternalInput")
    luid = nc.dram_tensor("luid", [9, P], BF16, kind="ExternalInput")
    lnid = nc.dram_tensor("lnid", [9, P], BF16, kind="ExternalInput")
    lrhd = nc.dram_tensor("lrhd", [P, P], BF16, kind="ExternalInput")
    luhd = nc.dram_tensor("luhd", [P, P], BF16, kind="ExternalInput")
    lnhd = nc.dram_tensor("lnhd", [P, P], BF16, kind="ExternalInput")
    lwd = nc.dram_tensor("lwd", [P, 2 * G], BF16, kind="ExternalInput")
    bnhd = nc.dram_tensor("bnhd", [P, 1], FP32, kind="ExternalInput")
    bwd = nc.dram_tensor("bwd", [2 * G, 1], FP32, kind="ExternalInput")
    wpinit = nc.dram_tensor("wpinit", [9, BT], BF16, kind="ExternalInput")
    outd = nc.dram_tensor("outd", [nsteps, NM, 2 * G, BT], BF16,
                          kind="ExternalOutput")

    xv = xT.rearrange("(kb p) b -> p kb b", p=P)              # [128, 2, BC]

    with TileContext(nc) as tc:
        with (
            tc.tile_pool(name="const", bufs=1) as const,
            tc.tile_pool(name="state", bufs=1) as state,
            tc.tile_pool(name="xp", bufs=3) as xp,
            tc.tile_pool(name="hp", bufs=2) as hp,
            tc.tile_pool(name="rup", bufs=3) as rup,
            tc.tile_pool(name="ntp", bufs=2) as ntp,
            tc.tile_pool(name="p1p", bufs=2) as p1p,
            tc.tile_pool(name="p2p", bufs=3) as p2p,
            tc.tile_pool(name="dp", bufs=2) as dp,
            tc.tile_pool(name="mps", bufs=2, space="PSUM") as mps,
            tc.tile_pool(name="zps", bufs=1, space="PSUM") as zps,
            tc.tile_pool(name="gps", bufs=1, space="PSUM") as gps,
            tc.tile_pool(name="nhp", bufs=1, space="PSUM") as nhp,
            tc.tile_pool(name="nip", bufs=1, space="PSUM") as nip,
            tc.tile_pool(name="wpp", bufs=1, space="PSUM") as wpp,
        ):
            w1s = const.tile([P, 2, H], BF16)
            nc.sync.dma_start(w1s[:], w1t.rearrange("(kb p) f -> p kb f", p=P))
            b1s = const.tile([P, 4], FP32)
            nc.sync.dma_start(b1s[:], b1d[:])
            w2s = const.tile([P, 4, H], BF16)
            nc.sync.dma_start(w2s[:], w2t.rearrange("(kb p) f -> p kb f", p=P))
            b2s = const.tile([P, 4], FP32)
            nc.sync.dma_start(b2s[:], b2d[:])
            wms = const.tile([P, 4, A], BF16)
            nc.sync.dma_start(wms[:], wmt.rearrange("(kb p) f -> p kb f", p=P))
            bms = const.tile([P, 1], FP32)
            nc.sync.dma_start(bms[:], bmd[:])

            def load_gru_consts():
                lris = const.tile([9, P], BF16, name="lris")
                nc.sync.dma_start(lris[:], lrid[:])
                luis = const.tile([9, P], BF16, name="luis")
                nc.sync.dma_start(luis[:], luid[:])
                lnis = const.tile([9, P], BF16, name="lnis")
                nc.sync.dma_start(lnis[:], lnid[:])
                lrhs = const.tile([P, P], BF16, name="lrhs")
                nc.sync.dma_start(lrhs[:], lrhd[:])
                luhs = const.tile([P, P], BF16, name="luhs")
                nc.sync.dma_start(luhs[:], luhd[:])
                lnhs = const.tile([P, P], BF16, name="lnhs")
                nc.sync.dma_start(lnhs[:], lnhd[:])
                lws = const.tile([P, 2 * G], BF16, name="lws")
                nc.sync.dma_start(lws[:], lwd[:])
                bnhs = const.tile([P, 1], FP32, name="bnhs")
                nc.sync.dma_start(bnhs[:], bnhd[:])
                bws = const.tile([2 * G, 1], FP32, name="bws")
                nc.sync.dma_start(bws[:], bwd[:])
                wps = []
                for m in range(NM):
                    wt = state.tile([9, BT], BF16, tag=f"WP{m}", name=f"WP{m}")
                    wps.append(wt)
                    nc.sync.dma_start(wt[:], wpinit[:])
                return lris, luis, lnis, lrhs, luhs, lnhs, lws, bnhs, bws, wps

            Z = []
            for m in range(NM):
                zt = state.tile([P, BT], BF16, tag=f"Z{m}", name=f"Z{m}")
                Z.append(zt)

            # ------------- emission helpers -------------
            psZs = [None] * NM

            def emit_mlp_group(m, g):
                c0 = m * MACRO + g * BT
                X = xp.tile([P, 2, BT], BF16, tag="X", name="X")
                nc.sync.dma_start(X[:], xv[:, :, c0:c0 + BT])
                H1 = hp.tile([P, 4, BT], BF16, tag="H1", name="H1")
                for f in range(4):
                    ps = mps.tile([P, BT], FP32, tag="mm", name="ps")
                    nc.tensor.matmul(ps[:], w1s[:, 0, f * P:(f + 1) * P],
                                     X[:, 0, :], start=True, stop=False)
                    nc.tensor.matmul(ps[:], w1s[:, 1, f * P:(f + 1) * P],
                                     X[:, 1, :], start=False, stop=True)
                    if f % 2 == 0:
                        nc.scalar.activation(H1[:, f, :], ps[:], AF.Relu,
                                             bias=b1s[:, f:f + 1])
                    else:
                        nc.vector.tensor_scalar(H1[:, f, :], ps[:],
                                                b1s[:, f:f + 1], 0.0,
                                                OP.add, OP.max)
                H2 = hp.tile([P, 4, BT], BF16, tag="H2", name="H2")
                for f in range(4):
                    ps = mps.tile([P, BT], FP32, tag="mm", name="ps")
                    for k in range(4):
                        nc.tensor.matmul(ps[:], w2s[:, k, f * P:(f + 1) * P],
                                         H1[:, k, :], start=(k == 0),
                                         stop=(k == 3))
                    if f % 2 == 0:
                        nc.scalar.activation(H2[:, f, :], ps[:], AF.Relu,
                                             bias=b2s[:, f:f + 1])
                    else:
                        nc.vector.tensor_scalar(H2[:, f, :], ps[:],
                                                b2s[:, f:f + 1], 0.0,
                                                OP.add, OP.max)
                if g == 0:
                    psZs[m] = zps.tile([P, BT], FP32, tag="psZ", name="psZ")
                for k in range(4):
                    nc.tensor.matmul(psZs[m][g * A:(g + 1) * A, :],
                                     wms[:, k, :], H2[:, k, :],
                                     start=(k == 0), stop=(k == 3),
                                     tile_position=(0, g * A))
                if g == G - 1:
                    nc.scalar.activation(Z[m][:], psZs[m][:], AF.Identity,
                                         bias=bms[:, 0:1])

            # GRU iteration state, staged across issue-slots:
            #  A(q): gate matmuls, r/u/v sigmoids, P1/P2, M2 = u*z_old
            #  B(q): NT tanh, M1 = v*nt, z = M1 + M2, psW   (1 slot later)
            #  C(q): wp accumulate + output DMA             (2 slots later)
            class It:
                __slots__ = ("t", "m", "psRU", "psNH", "psNI", "R", "U", "V",
                             "P1", "P2", "M2", "NT", "psW", "pool_m1")

            def emit_A_gates(q):
                m = q.m
                q.psRU = gps.tile([P, 2 * BT], FP32, tag="psRU", name="psRU")
                nc.tensor.matmul(q.psRU[:, 0:BT], lris[:], WPS[m][:],
                                 start=True, stop=False)
                nc.tensor.matmul(q.psRU[:, 0:BT], lrhs[:], Z[m][:],
                                 start=False, stop=True)
                nc.tensor.matmul(q.psRU[:, BT:2 * BT], luis[:], WPS[m][:],
                                 start=True, stop=False)
                nc.tensor.matmul(q.psRU[:, BT:2 * BT], luhs[:], Z[m][:],
                                 start=False, stop=True)
                q.psNH = nhp.tile([P, BT], FP32, tag="psNH", name="psNH")
                nc.tensor.matmul(q.psNH[:], lnhs[:], Z[m][:],
                                 start=True, stop=True)
                q.psNI = nip.tile([P, BT], FP32, tag="psNI", name="psNI")
                nc.tensor.matmul(q.psNI[:], lnis[:], WPS[m][:],
                                 start=True, stop=True)

            def emit_A_rest(q):
                # Act: r and u sigmoids, v = 1-u = sigmoid(-pre_u)
                q.R = rup.tile([P, BT], BF16, tag="R", name="R")
                nc.scalar.activation(q.R[:], q.psRU[:, 0:BT], AF.Sigmoid)
                q.U = rup.tile([P, BT], BF16, tag="U", name="U")
                nc.scalar.activation(q.U[:], q.psRU[:, BT:2 * BT], AF.Sigmoid)
                q.V = rup.tile([P, BT], BF16, tag="V", name="V")
                nc.scalar.activation(q.V[:], q.psRU[:, BT:2 * BT], AF.Sigmoid,
                                     scale=-1.0)
                # DVE: n-gate pre-activation from PSUM
                q.P1 = p1p.tile([P, BT], FP32, tag="P1", name="P1")
                nc.vector.scalar_tensor_tensor(q.P1[:], q.psNH[:],
                                               bnhs[:, 0:1], q.R[:],
                                               OP.add, OP.mult)
                q.P2 = p2p.tile([P, BT], FP32, tag="P2", name="P2")
                nc.vector.scalar_tensor_tensor(q.P2[:], q.psNI[:], 0.0,
                                               q.P1[:], OP.add, OP.add)
                # Pool (off critical path): m2 = u * z_old
                q.M2 = dp.tile([P, BT], BF16, tag="M2", name="M2")
                nc.gpsimd.tensor_tensor(q.M2[:], q.U[:], Z[q.m][:], OP.mult)

            def emit_B(q):
                m = q.m
                q.NT = ntp.tile([P, BT], BF16, tag="NT", name="NT")
                nc.scalar.activation(q.NT[:], q.P2[:], AF.Tanh)
                M1 = dp.tile([P, BT], BF16, tag="M1", name="M1")
                if q.pool_m1:
                    nc.gpsimd.tensor_tensor(M1[:], q.V[:], q.NT[:], OP.mult)
                else:
                    nc.vector.scalar_tensor_tensor(M1[:], q.V[:], 0.0,
                                                   q.NT[:], OP.add, OP.mult)
                nc.vector.scalar_tensor_tensor(Z[m][:], M1[:], 0.0, q.M2[:],
                                               OP.add, OP.add)

            def emit_C(q):
                m = q.m
                q.psW = wpp.tile([2 * G, BT], FP32, tag="psW", name="psW")
                nc.tensor.matmul(q.psW[:], lws[:], Z[m][:],
                                 start=True, stop=True)
                nc.vector.scalar_tensor_tensor(WPS[m][0:8, :], q.psW[:],
                                               bws[:, 0:1], WPS[m][0:8, :],
                                               OP.add, OP.add)
                nc.sync.dma_start(outd[q.t, m, :, :], WPS[m][0:8, :])

            # ------------- merged list schedule -------------
            next_t = [0] * NM
            last_slot = [-10] * NM
            mlp_done = [False] * NM
            # hold back late macros during the merged phase so the drain tail
            # keeps several macros in rotation (hides recurrence latency)
            caps = [nsteps, nsteps, nsteps, nsteps, nsteps,
                    nsteps - 3, nsteps - 6, nsteps]
            merged_phase = [False]
            b_queue = []        # (slot_of_A, It)
            c_queue = []        # (slot_of_B, It)
            slot = 0

            def gru_slot():
                nonlocal slot
                # pop due C first (ready PE/DVE work heads the queues)
                if c_queue and c_queue[0][0] <= slot - 1:
                    emit_C(c_queue.pop(0)[1])
                # new A: greedy pick ready macro with most remaining steps
                pick = -1
                best = 0
                for m in range(NM):
                    if (mlp_done[m] and next_t[m] < caps[m]
                            and last_slot[m] <= slot - 2):
                        rem = nsteps - next_t[m]
                        if rem > best:
                            best = rem
                            pick = m
                qa = None
                if pick >= 0:
                    qa = It()
                    qa.t = next_t[pick]
                    qa.m = pick
                    qa.pool_m1 = merged_phase[0]
                    next_t[pick] += 1
                    last_slot[pick] = slot
                    emit_A_gates(qa)
                # due B (deps all from earlier slots -> engines see ready work)
                if b_queue and b_queue[0][0] <= slot - 1:
                    qb = b_queue.pop(0)[1]
                    emit_B(qb)
                    c_queue.append((slot, qb))
                if qa is not None:
                    emit_A_rest(qa)
                    b_queue.append((slot, qa))
                slot += 1

            for s in range(NM * G):
                emit_mlp_group(s // G, s % G)
                if s == 0:
                    (lris, luis, lnis, lrhs, luhs, lnhs, lws, bnhs, bws,
                     WPS) = load_gru_consts()
                if s % G == G - 1:
                    mlp_done[s // G] = True
                for _ in range(3):
                    gru_slot()
            caps = [nsteps] * NM
            merged_phase[0] = False
            while (any(next_t[m] < nsteps for m in range(NM))
                   or b_queue or c_queue):
                gru_slot()
    nc.compile()
    return nc


LAST_RESULT = None


def kernel(**inputs) -> np.ndarray:
    global LAST_RESULT
    x = np.asarray(inputs["x"], np.float32)
    W1 = np.asarray(inputs["W1"], np.float32)
    b1 = np.asarray(inputs["b1"], np.float32)
    W2 = np.asarray(inputs["W2"], np.float32)
    b2 = np.asarray(inputs["b2"], np.float32)
    Wm = np.asarray(inputs["Wm"], np.float32)
    bm = np.asarray(inputs["bm"], np.float32)
    w_ih = np.asarray(inputs["w_ih"], np.float32)
    w_hh = np.asarray(inputs["w_hh"], np.float32)
    b_ih = np.asarray(inputs["b_ih"], np.float32)
    b_hh = np.asarray(inputs["b_hh"], np.float32)
    Ww = np.asarray(inputs["Ww"], np.float32)
    bw = np.asarray(inputs["bw"], np.float32)
    T = int(inputs["pred_length"])

    I4 = np.eye(G, dtype=np.float32)

    def pack9(wg, bias):
        # [9, 128]: rows 0-7 block-diag i-weights, row 8 the folded bias
        mret = np.zeros((9, P), np.float32)
        mret[0:8, :] = np.kron(I4, wg.T)
        mret[8, :] = np.tile(bias, G)
        return mret.astype(BF)

    common = {
        "w1t": W1.T.astype(BF),
        "w2t": W2.T.astype(BF),
        "wmt": Wm.T.astype(BF),
        "b1d": np.ascontiguousarray(b1.reshape(4, P).T),
        "b2d": np.ascontiguousarray(b2.reshape(4, P).T),
        "bmd": np.tile(bm, G).reshape(P, 1).copy(),
        "lrid": pack9(w_ih[0:A], b_ih[0:A] + b_hh[0:A]),
        "luid": pack9(w_ih[A:2 * A], b_ih[A:2 * A] + b_hh[A:2 * A]),
        "lnid": pack9(w_ih[2 * A:3 * A], b_ih[2 * A:3 * A]),
        "lrhd": np.kron(I4, w_hh[0:A].T).astype(BF),
        "luhd": np.kron(I4, w_hh[A:2 * A].T).astype(BF),
        "lnhd": np.kron(I4, w_hh[2 * A:3 * A].T).astype(BF),
        "lwd": np.kron(I4, Ww.T).astype(BF),
        "bnhd": np.tile(b_hh[2 * A:3 * A], G).reshape(P, 1).copy(),
        "bwd": np.tile(bw, G).reshape(2 * G, 1).copy(),
        "wpinit": np.concatenate(
            [np.zeros((8, BT), np.float32), np.ones((1, BT), np.float32)]
        ).astype(BF),
    }
    xTb = x.T.astype(BF)                     # [S, B]
    in_maps = []
    for i in range(NCORES):
        m = dict(common)
        m["xT"] = np.ascontiguousarray(xTb[:, i * BC:(i + 1) * BC])
        in_maps.append(m)

    if T not in _CACHE:
        _CACHE[T] = _build(T)
    nc = _CACHE[T]
    res = run_bass_kernel_spmd(nc, in_maps, core_ids=list(range(NCORES)))
    LAST_RESULT = res
    parts = []
    for i in range(NCORES):
        o = np.asarray(res.results[i]["outd"]).astype(np.float32)
        # [T, NM, 2G, BT] -> rows m*2048 + g*512 + c, cols 2t+j
        o = o.reshape(T, NM, G, 2, BT).transpose(1, 2, 4, 0, 3)
        parts.append(o.reshape(BC, 2 * T))
    return np.ascontiguousarray(np.concatenate(parts, axis=0))


# revision 29
# speedup vs baseline: 1.2626x; 1.0025x over previous
import sys

sys.path.insert(0, "/opt/trn_rl_repo")

import numpy as np
import ml_dtypes

import concourse.bass as bass
import concourse.bacc as bacc
import concourse.mybir as mybir
from concourse.tile import TileContext
from concourse.bass_utils import run_bass_kernel_spmd

P = 128          # partitions
BT = 512         # batch-tile (free dim)
G = 4            # batch groups packed into 128 partitions for the GRU
NCORES = 8
B, S, H, A = 131072, 256, 512, 32
BC = B // NCORES           # 16384 rows per core
MACRO = G * BT             # 2048 rows per GRU macro-tile
NM = BC // MACRO           # 8 macro-tiles per core

FP32 = mybir.dt.float32
BF16 = mybir.dt.bfloat16
AF = mybir.ActivationFunctionType
OP = mybir.AluOpType
BF = ml_dtypes.bfloat16

_CACHE = {}


def _build(nsteps: int) -> bass.Bass:
    nc = bacc.Bacc("TRN2", target_bir_lowering=False, debug=False,
                   num_devices=NCORES)

    xT = nc.dram_tensor("xT", [S, BC], BF16, kind="ExternalInput")
    w1t = nc.dram_tensor("w1t", [S, H], BF16, kind="ExternalInput")
    w2t = nc.dram_tensor("w2t", [H, H], BF16, kind="ExternalInput")
    wmt = nc.dram_tensor("wmt", [H, A], BF16, kind="ExternalInput")
    b1d = nc.dram_tensor("b1d", [P, 4], FP32, kind="ExternalInput")
    b2d = nc.dram_tensor("b2d", [P, 4], FP32, kind="ExternalInput")
    bmd = nc.dram_tensor("bmd", [P, 1], FP32, kind="ExternalInput")
    # i-side gate weights with bias folded in via a constant-1 ninth row
    lrid = nc.dram_tensor("lrid", [9, P], BF16, kind="ExternalInput")
    luid = nc.dram_tensor("luid", [9, P], BF16, kind="ExternalInput")
    lnid = nc.dram_tensor("lnid", [9, P], BF16, kind="ExternalInput")
    lrhd = nc.dram_tensor("lrhd", [P, P], BF16, kind="ExternalInput")
    luhd = nc.dram_tensor("luhd", [P, P], BF16, kind="ExternalInput")
    lnhd = nc.dram_tensor("lnhd", [P, P], BF16, kind="ExternalInput")
    lwd = nc.dram_tensor("lwd", [P, 2 * G], BF16, kind="ExternalInput")
    bnhd = nc.dram_tensor("bnhd", [P, 1], FP32, kind="ExternalInput")
    bwd = nc.dram_tensor("bwd", [2 * G, 1], FP32, kind="ExternalInput")
    wpinit = nc.dram_tensor("wpinit", [9, BT], BF16, kind="ExternalInput")
    outd = nc.dram_tensor("outd", [nsteps, NM, 2 * G, BT], BF16,
                          kind="ExternalOutput")

    xv = xT.rearrange("(kb p) b -> p kb b", p=P)              # [128, 2, BC]

    with TileContext(nc) as tc:
        with (
            tc.tile_pool(name="const", bufs=1) as const,
            tc.tile_pool(name="state", bufs=1) as state,
            tc.tile_pool(name="xp", bufs=3) as xp,
            tc.tile_pool(name="hp", bufs=2) as hp,
            tc.tile_pool(name="rup", bufs=3) as rup,
            tc.tile_pool(name="ntp", bufs=2) as ntp,
            tc.tile_pool(name="p1p", bufs=2) as p1p,
            tc.tile_pool(name="p2p", bufs=3) as p2p,
            tc.tile_pool(name="dp", bufs=2) as dp,
            tc.tile_pool(name="mps", bufs=2, space="PSUM") as mps,
            tc.tile_pool(name="gps", bufs=1, space="PSUM") as gps,
            tc.tile_pool(name="nhp", bufs=1, space="PSUM") as nhp,
            tc.tile_pool(name="nip", bufs=2, space="PSUM") as nip,
            tc.tile_pool(name="wpp", bufs=1, space="PSUM") as wpp,
        ):
            w1s = const.tile([P, 2, H], BF16)
            nc.sync.dma_start(w1s[:], w1t.rearrange("(kb p) f -> p kb f", p=P))
            b1s = const.tile([P, 4], FP32)
            nc.sync.dma_start(b1s[:], b1d[:])
            w2s = const.tile([P, 4, H], BF16)
            nc.sync.dma_start(w2s[:], w2t.rearrange("(kb p) f -> p kb f", p=P))
            b2s = const.tile([P, 4], FP32)
            nc.sync.dma_start(b2s[:], b2d[:])
            wms = const.tile([P, 4, A], BF16)
            nc.sync.dma_start(wms[:], wmt.rearrange("(kb p) f -> p kb f", p=P))
            bms = const.tile([P, 1], FP32)
            nc.sync.dma_start(bms[:], bmd[:])

            def load_gru_consts():
                lris = const.tile([9, P], BF16, name="lris")
                nc.sync.dma_start(lris[:], lrid[:])
                luis = const.tile([9, P], BF16, name="luis")
                nc.sync.dma_start(luis[:], luid[:])
                lnis = const.tile([9, P], BF16, name="lnis")
                nc.sync.dma_start(lnis[:], lnid[:])
                lrhs = const.tile([P, P], BF16, name="lrhs")
                nc.sync.dma_start(lrhs[:], lrhd[:])
                luhs = const.tile([P, P], BF16, name="luhs")
                nc.sync.dma_start(luhs[:], luhd[:])
                lnhs = const.tile([P, P], BF16, name="lnhs")
                nc.sync.dma_start(lnhs[:], lnhd[:])
                lws = const.tile([P, 2 * G], BF16, name="lws")
                nc.sync.dma_start(lws[:], lwd[:])
                bnhs = const.tile([P, 1], FP32, name="bnhs")
                nc.sync.dma_start(bnhs[:], bnhd[:])
                bws = const.tile([2 * G, 1], FP32, name="bws")
                nc.sync.dma_start(bws[:], bwd[:])
                wps = []
                for m in range(NM):
                    wt = state.tile([9, BT], BF16, tag=f"WP{m}", name=f"WP{m}")
                    wps.append(wt)
                    nc.sync.dma_start(wt[:], wpinit[:])
                return lris, luis, lnis, lrhs, luhs, lnhs, lws, bnhs, bws, wps

            Z = []
            for m in range(NM):
                zt = state.tile([P, BT], BF16, tag=f"Z{m}", name=f"Z{m}")
                Z.append(zt)

            # ------------- emission helpers -------------
            H2s = [[None] * G for _ in range(NM)]

            def emit_mlp_group(m, g):
                c0 = m * MACRO + g * BT
                X = xp.tile([P, 2, BT], BF16, tag="X", name="X")
                nc.sync.dma_start(X[:], xv[:, :, c0:c0 + BT])
                H1 = hp.tile([P, 4, BT], BF16, tag="H1", name="H1")
                for f in range(4):
                    ps = mps.tile([P, BT], FP32, tag="mm", name="ps")
                    nc.tensor.matmul(ps[:], w1s[:, 0, f * P:(f + 1) * P],
                                     X[:, 0, :], start=True, stop=False)
                    nc.tensor.matmul(ps[:], w1s[:, 1, f * P:(f + 1) * P],
                                     X[:, 1, :], start=False, stop=True)
                    if f % 2 == 0:
                        nc.scalar.activation(H1[:, f, :], ps[:], AF.Relu,
                                             bias=b1s[:, f:f + 1])
                    else:
                        nc.vector.tensor_scalar(H1[:, f, :], ps[:],
                                                b1s[:, f:f + 1], 0.0,
                                                OP.add, OP.max)
                H2 = hp.tile([P, 4, BT], BF16, tag="H2", name="H2", bufs=5)
                for f in range(4):
                    ps = mps.tile([P, BT], FP32, tag="mm", name="ps")
                    for k in range(4):
                        nc.tensor.matmul(ps[:], w2s[:, k, f * P:(f + 1) * P],
                                         H1[:, k, :], start=(k == 0),
                                         stop=(k == 3))
                    if f % 2 == 0:
                        nc.scalar.activation(H2[:, f, :], ps[:], AF.Relu,
                                             bias=b2s[:, f:f + 1])
                    else:
                        nc.vector.tensor_scalar(H2[:, f, :], ps[:],
                                                b2s[:, f:f + 1], 0.0,
                                                OP.add, OP.max)
                H2s[m][g] = H2
                if g == G - 1:
                    # deferred z-projection: one 16-matmul burst per macro,
                    # reusing the mm ring (no dedicated psZ bank)
                    psZ = mps.tile([P, BT], FP32, tag="mm", name="psZ")
                    for g2 in range(G):
                        for k in range(4):
                            nc.tensor.matmul(psZ[g2 * A:(g2 + 1) * A, :],
                                             wms[:, k, :],
                                             H2s[m][g2][:, k, :],
                                             start=(k == 0), stop=(k == 3),
                                             tile_position=(0, g2 * A))
                    nc.scalar.activation(Z[m][:], psZ[:], AF.Identity,
                                         bias=bms[:, 0:1])

            # GRU iteration state, staged across issue-slots:
            #  A(q): gate matmuls, r/u/v sigmoids, P1/P2, M2 = u*z_old
            #  B(q): NT tanh, M1 = v*nt, z = M1 + M2, psW   (1 slot later)
            #  C(q): wp accumulate + output DMA             (2 slots later)
            class It:
                __slots__ = ("t", "m", "psRU", "psNH", "psNI", "R", "U", "V",
                             "P1", "P2", "M2", "NT", "psW", "pool_m1")

            def emit_A_gates(q):
                m = q.m
                q.psRU = gps.tile([P, 2 * BT], FP32, tag="psRU", name="psRU")
                nc.tensor.matmul(q.psRU[:, 0:BT], lris[:], WPS[m][:],
                                 start=True, stop=False)
                nc.tensor.matmul(q.psRU[:, 0:BT], lrhs[:], Z[m][:],
                                 start=False, stop=True)
                nc.tensor.matmul(q.psRU[:, BT:2 * BT], luis[:], WPS[m][:],
                                 start=True, stop=False)
                nc.tensor.matmul(q.psRU[:, BT:2 * BT], luhs[:], Z[m][:],
                                 start=False, stop=True)
                q.psNH = nhp.tile([P, BT], FP32, tag="psNH", name="psNH")
                nc.tensor.matmul(q.psNH[:], lnhs[:], Z[m][:],
                                 start=True, stop=True)
                q.psNI = nip.tile([P, BT], FP32, tag="psNI", name="psNI")
                nc.tensor.matmul(q.psNI[:], lnis[:], WPS[m][:],
                                 start=True, stop=True)

            def emit_A_rest(q):
                # Act: r and u sigmoids, v = 1-u = sigmoid(-pre_u)
                q.R = rup.tile([P, BT], BF16, tag="R", name="R")
                nc.scalar.activation(q.R[:], q.psRU[:, 0:BT], AF.Sigmoid)
                q.U = rup.tile([P, BT], BF16, tag="U", name="U")
                nc.scalar.activation(q.U[:], q.psRU[:, BT:2 * BT], AF.Sigmoid)
                q.V = rup.tile([P, BT], BF16, tag="V", name="V")
                nc.scalar.activation(q.V[:], q.psRU[:, BT:2 * BT], AF.Sigmoid,
                                     scale=-1.0)
                # DVE: n-gate pre-activation from PSUM
                q.P1 = p1p.tile([P, BT], FP32, tag="P1", name="P1")
                nc.vector.scalar_tensor_tensor(q.P1[:], q.psNH[:],
                                               bnhs[:, 0:1], q.R[:],
                                               OP.add, OP.mult)
                q.P2 = p2p.tile([P, BT], FP32, tag="P2", name="P2")
                nc.vector.scalar_tensor_tensor(q.P2[:], q.psNI[:], 0.0,
                                               q.P1[:], OP.add, OP.add)
                # Pool (off critical path): m2 = u * z_old
                q.M2 = dp.tile([P, BT], BF16, tag="M2", name="M2")
                nc.gpsimd.tensor_tensor(q.M2[:], q.U[:], Z[q.m][:], OP.mult)

            def emit_B(q):
                m = q.m
                q.NT = ntp.tile([P, BT], BF16, tag="NT", name="NT")
                nc.scalar.activation(q.NT[:], q.P2[:], AF.Tanh)
                M1 = dp.tile([P, BT], BF16, tag="M1", name="M1")
                if q.pool_m1:
                    nc.gpsimd.tensor_tensor(M1[:], q.V[:], q.NT[:], OP.mult)
                else:
                    nc.vector.scalar_tensor_tensor(M1[:], q.V[:], 0.0,
                                                   q.NT[:], OP.add, OP.mult)
                nc.vector.scalar_tensor_tensor(Z[m][:], M1[:], 0.0, q.M2[:],
                                               OP.add, OP.add)

            def emit_C(q):
                m = q.m
                q.psW = wpp.tile([2 * G, BT], FP32, tag="psW", name="psW")
                nc.tensor.matmul(q.psW[:], lws[:], Z[m][:],
                                 start=True, stop=True)
                nc.vector.scalar_tensor_tensor(WPS[m][0:8, :], q.psW[:],
                                               bws[:, 0:1], WPS[m][0:8, :],
                                               OP.add, OP.add)
                nc.sync.dma_start(outd[q.t, m, :, :], WPS[m][0:8, :])

            # ------------- merged list schedule -------------
            next_t = [0] * NM
            last_slot = [-10] * NM
            mlp_done = [False] * NM
            # hold back late macros during the merged phase so the drain tail
            # keeps several macros in rotation (hides recurrence latency)
            caps = [nsteps, nsteps, nsteps, nsteps, nsteps,
                    nsteps - 3, nsteps - 6, nsteps]
            merged_phase = [False]
            b_queue = []        # (slot_of_A, It)
            c_queue = []        # (slot_of_B, It)
            slot = 0

            def gru_slot():
                nonlocal slot
                # pop due C first (ready PE/DVE work heads the queues)
                if c_queue and c_queue[0][0] <= slot - 1:
                    emit_C(c_queue.pop(0)[1])
                # new A: greedy pick ready macro with most remaining steps
                pick = -1
                best = 0
                for m in range(NM):
                    if (mlp_done[m] and next_t[m] < caps[m]
                            and last_slot[m] <= slot - 2):
                        rem = nsteps - next_t[m]
                        if rem > best:
                            best = rem
                            pick = m
                qa = None
                if pick >= 0:
                    qa = It()
                    qa.t = next_t[pick]
                    qa.m = pick
                    qa.pool_m1 = merged_phase[0]
                    next_t[pick] += 1
                    last_slot[pick] = slot
                    emit_A_gates(qa)
                # due B (deps all from earlier slots -> engines see ready work)
                if b_queue and b_queue[0][0] <= slot - 1:
                    qb = b_queue.pop(0)[1]
                    emit_B(qb)
                    c_queue.append((slot, qb))
                if qa is not None:
                    emit_A_rest(qa)
                    b_queue.append((slot, qa))
                slot += 1

            for s in range(NM * G):
                emit_mlp_group(s // G, s % G)
                if s == 0:
                    (lris, luis, lnis, lrhs, luhs, lnhs, lws, bnhs, bws,
                     WPS) = load_gru_consts()
                if s % G == G - 1:
                    mlp_done[s // G] = True
                for _ in range(3):
                    gru_slot()
            caps = [nsteps] * NM
            merged_phase[0] = False
            while (any(next_t[m] < nsteps for m in range(NM))
                   or b_queue or c_queue):
                gru_slot()
    nc.compile()
    return nc


LAST_RESULT = None


def kernel(**inputs) -> np.ndarray:
    global LAST_RESULT
    x = np.asarray(inputs["x"], np.float32)
    W1 = np.asarray(inputs["W1"], np.float32)
    b1 = np.asarray(inputs["b1"], np.float32)
    W2 = np.asarray(inputs["W2"], np.float32)
    b2 = np.asarray(inputs["b2"], np.float32)
    Wm = np.asarray(inputs["Wm"], np.float32)
    bm = np.asarray(inputs["bm"], np.float32)
    w_ih = np.asarray(inputs["w_ih"], np.float32)
    w_hh = np.asarray(inputs["w_hh"], np.float32)
    b_ih = np.asarray(inputs["b_ih"], np.float32)
    b_hh = np.asarray(inputs["b_hh"], np.float32)
    Ww = np.asarray(inputs["Ww"], np.float32)
    bw = np.asarray(inputs["bw"], np.float32)
    T = int(inputs["pred_length"])

    I4 = np.eye(G, dtype=np.float32)

    def pack9(wg, bias):
        # [9, 128]: rows 0-7 block-diag i-weights, row 8 the folded bias
        mret = np.zeros((9, P), np.float32)
        mret[0:8, :] = np.kron(I4, wg.T)
        mret[8, :] = np.tile(bias, G)
        return mret.astype(BF)

    common = {
        "w1t": W1.T.astype(BF),
        "w2t": W2.T.astype(BF),
        "wmt": Wm.T.astype(BF),
        "b1d": np.ascontiguousarray(b1.reshape(4, P).T),
        "b2d": np.ascontiguousarray(b2.reshape(4, P).T),
        "bmd": np.tile(bm, G).reshape(P, 1).copy(),
        "lrid": pack9(w_ih[0:A], b_ih[0:A] + b_hh[0:A]),
        "luid": pack9(w_ih[A:2 * A], b_ih[A:2 * A] + b_hh[A:2 * A]),
        "lnid": pack9(w_ih[2 * A:3 * A], b_ih[2 * A:3 * A]),
        "lrhd": np.kron(I4, w_hh[0:A].T).astype(BF),
        "luhd": np.kron(I4, w_hh[A:2 * A].T).astype(BF),
        "lnhd": np.kron(I4, w_hh[2 * A:3 * A].T).astype(BF),
        "lwd": np.kron(I4, Ww.T).astype(BF),
        "bnhd": np.tile(b_hh[2 * A:3 * A], G).reshape(P, 1).copy(),
        "bwd": np.tile(bw, G).reshape(2 * G, 1).copy(),
        "wpinit": np.concatenate(
            [np.zeros((8, BT), np.float32), np.ones((1, BT), np.float32)]
        ).astype(BF),
    }
    xTb = x.T.astype(BF)                     # [S, B]
    in_maps = []
    for i in range(NCORES):
        m = dict(common)
        m["xT"] = np.ascontiguousarray(xTb[:, i * BC:(i + 1) * BC])
        in_maps.append(m)

    if T not in _CACHE:
        _CACHE[T] = _build(T)
    nc = _CACHE[T]
    res = run_bass_kernel_spmd(nc, in_maps, core_ids=list(range(NCORES)))
    LAST_RESULT = res
    parts = []
    for i in range(NCORES):
        o = np.asarray(res.results[i]["outd"]).astype(np.float32)
        # [T, NM, 2G, BT] -> rows m*2048 + g*512 + c, cols 2t+j
        o = o.reshape(T, NM, G, 2, BT).transpose(1, 2, 4, 0, 3)
        parts.append(o.reshape(BC, 2 * T))
    return np.ascontiguousarray(np.concatenate(parts, axis=0))
